# revision 20
# baseline (speedup 1.0000x reference)
"""Trainium2 Bass kernel for nn_AMMaskedLinear (v6).

Math: the reference's per-sample weight mask is separable:
    present[b,v] = any_j(hidden_rank[b,j] == v)            (v in 0..32)
    pl[b,i] = present[b, r_low[i]]  & (r_low[i]  != 0)
    om[b,o] = present[b, r_high[o]]
    E[j,k]  = (r_high[k] >= max(r_low[j], 1)) * direction^T[j,k]
    Y[k,b]  = sum_j E[j,k] * pl[j,b] * x[j,b]
    out[k,b] = om[k,b] * (cscale_b[k] * Y[k,b] + cbias_b[k])

v6 over v5 (23.2us):
  - the E mask moves off DVE onto the otherwise-idle PE as a rank-33
    one-hot matmul into a BF16 psum (v4's f32 psum made the E multiply
    read at half rate): mask[j,k] = sum_v [max(r_low[j],1)==v]*[v<=
    r_high[k]].  The A'/Bm table builds slot into the pack right after
    big lands, so the PE mask matmuls fully overlap the pack tail and
    replace DVE's 1.3us IS_LE with ~0.65us of table ops.
  - w1/w2 shifts fused into one [128,2,512] tensor_tensor.
  - identity copy moved to ACT (was a 160ns DVE op + drain).
  - epilogue back to z = cs*Y + cb (stt) then out = z*om (one TT).

Presence pack (HW-verified shift-overflow-to-zero; probe: u16 << s == 0
for s >= 16): w0 = 1<<hr (values 0..15, no clamp), w1 = 1<<max(hr-15,0)
(16..30), w2 = 1<<max(hr-30,0) (31..32); natural value<->column c == v.
vv row 0 holds -1 on the r_low side so the pl gather's (r_low != 0)
comes from the table contents; both gathers contract rows 0:33.

Timing floors (measured): any 128-partition DMA's semaphore is gated by
SDMA engine 15's wake at ~10-11us from NEFF start; sync's qSP HWDGE ring
is serviced before qAct's, so all loads stay on sync.
"""

import numpy as np

B, IN, OUT, D = 64, 1024, 1024, 32
NCORES = 8
KSH = OUT // NCORES  # 128 outputs per core
NT = IN // 128       # 8 contraction tiles
NV = 33              # rank values 0..32

HRW = 256            # hrp [128, 256] f32 = [128, 512] u16

# big [128, GW] f32 — chain 2 on sync (tables + x; dirT split out so
# this chain lands ~1.3us earlier; dirT is not needed until E)
G_X = 0              # [128, 256]  xT bf16 [128, 8, 64]
G_CS = 256           # [128, 1]    cscale_b shard f32
G_CB = 257           # [128, 1]    cbias_b shard f32
G_MSK = 258          # [64, 17]    u16 extraction masks [64, 34]
G_IDT = 275          # [64, 32]    bf16 identity [64, 64]
G_VOM = 307          # [33, 1]     value-iota f32 (vom[c] = c)
G_POW = 308          # [33, 1]     2^-bit compensation factors f32
G_VV = 309           # [33, 576]   bf16 r_low||r_high bcast [33, 1152]
GW = 885
DW = 512             # dir [128, 512] f32 = dirT bf16 [128, 8, 128], chain 3

_cached = {}


def _build_nc():
    import contextlib

    import concourse.bass as bass
    import concourse.mybir as mybir

    f32 = mybir.dt.float32
    bf16 = mybir.dt.bfloat16
    u16 = mybir.dt.uint16
    Alu = mybir.AluOpType
    Act = mybir.ActivationFunctionType

    nc = bass.Bass()

    hrp_h = nc.declare_dram_parameter("hrp", [128, HRW], f32, isOutput=False)
    big_h = nc.declare_dram_parameter("big", [128, GW], f32, isOutput=False)
    dir_h = nc.declare_dram_parameter("dir", [128, DW], f32, isOutput=False)
    out_h = nc.declare_dram_parameter("out", [KSH, B], f32, isOutput=True)

    ctx = contextlib.ExitStack()

    def sb(name, shape, dt=f32):
        return ctx.enter_context(nc.sbuf_tensor(name, shape, dt))[:]

    def ps(name, shape, dt=f32):
        return ctx.enter_context(nc.psum_tensor(name, shape, dt))[:]

    with ctx:
        hrp_t = sb("hrp_t", [128, HRW])
        big_t = sb("big_t", [128, GW])
        dir_t = sb("dir_t", [128, DW])
        ident_t = sb("ident_t", [64, 64], bf16)

        ones_t = sb("ones_t", [128, 512], u16)
        amt_t = sb("amt_t", [128, 2, 512], u16)
        w_t = sb("w_t", [128, 3, 512], u16)
        packed_t = sb("packed_t", [128, 3], u16)
        packhi_t = sb("packhi_t", [64, 3], u16)
        p33u_t = sb("p33u_t", [64, NV], u16)
        p33_t = sb("p33_t", [64, NV], bf16)
        presT_t = sb("presT_t", [NV, 64], bf16)
        oh_t = sb("oh_t", [NV, IN + KSH], bf16)
        Ap_t = sb("Ap_t", [NV, IN], bf16)
        Bm_t = sb("Bm_t", [NV, KSH], bf16)
        E_t = sb("E_t", [128, NT, KSH], bf16)
        mask_sb = sb("mask_sb", [128, NT, KSH], bf16)
        xlT_t = sb("xlT_t", [128, NT, B], bf16)
        z_t = sb("z_t", [KSH, B])
        outT_t = sb("outT_t", [KSH, B])
        warm_t = sb("warm_t", [128, 1])

        presT_ps = ps("presT_ps", [NV, 64], bf16)
        mask_ps = ps("mask_ps", [128, NT, KSH])
        plT_ps = ps("plT_ps", [128, NT, B])
        omT_ps = ps("omT_ps", [KSH, B])
        Y_ps = ps("Y_ps", [KSH, B])

        hr_ap = hrp_t[:, :].bitcast(u16)                           # [128, 512]
        cs_ap = big_t[:, G_CS : G_CS + 1]
        cb_ap = big_t[:, G_CB : G_CB + 1]
        msk_ap = big_t[0:64, G_MSK : G_MSK + 17].bitcast(u16)      # [64, 34]
        ident_ap = big_t[0:64, G_IDT : G_IDT + 32].bitcast(bf16)
        vom_ap = big_t[0:NV, G_VOM : G_VOM + 1]
        pow_ap = big_t[0:NV, G_POW : G_POW + 1]
        vv_ap = big_t[0:NV, G_VV : G_VV + (IN + KSH) // 2].bitcast(bf16)
        xT_ap = big_t[:, G_X : G_X + 256].bitcast(bf16).rearrange(
            "p (t b) -> p t b", t=NT
        )
        dirT_ap = dir_t[:, :].bitcast(bf16).rearrange(
            "p (t k) -> p t k", t=NT
        )

        hr_sem = ctx.enter_context(nc.semaphore("hr_sem"))
        big_sem = ctx.enter_context(nc.semaphore("big_sem"))
        dir_sem = ctx.enter_context(nc.semaphore("dir_sem"))
        out_sem = ctx.enter_context(nc.semaphore("out_sem"))
        dve_sem = ctx.enter_context(nc.semaphore("dve_sem"))
        pe_sem = ctx.enter_context(nc.semaphore("pe_sem"))
        act_sem = ctx.enter_context(nc.semaphore("act_sem"))
        block = ctx.enter_context(nc.Block())

        @block.sync
        def _(sync):
            # ALL dmas on sync: the qAct ring is serviced after qSP's
            sync.dma_start(out=hrp_t, in_=hrp_h[:, :]).then_inc(hr_sem, 16)
            sync.dma_start(out=big_t, in_=big_h[:, :]).then_inc(big_sem, 16)
            sync.dma_start(out=dir_t, in_=dir_h[:, :]).then_inc(dir_sem, 16)
            sync.wait_ge(dve_sem, 7)
            sync.dma_start(out=out_h[:, :], in_=outT_t).then_inc(out_sem, 16)
            sync.wait_ge(out_sem, 16)  # required: NEFF exit must not race the store


        @block.scalar
        def _(scalar):
            # warm the activation table in the DMA shadow (content irrelevant)
            nc.scalar.activation(
                out=warm_t, in_=big_t[:, 0:1], func=Act.Identity,
                bias=0.0, scale=1.0,
            )
            scalar.wait_ge(big_sem, 16)
            nc.scalar.activation(
                out=ident_t, in_=ident_ap, func=Act.Copy, bias=0.0, scale=1.0,
            ).then_inc(act_sem, 1)
            # act=1: ident in SBUF (PE transpose)
            scalar.wait_ge(pe_sem, 1)
            nc.scalar.activation(
                out=mask_sb[:, 0 : NT // 2, :], in_=mask_ps[:, 0 : NT // 2, :],
                func=Act.Copy, bias=0.0, scale=1.0,
            ).then_inc(act_sem, 1)
            # act=2: mask half 1 in SBUF
            scalar.wait_ge(pe_sem, 2)
            nc.scalar.activation(
                out=mask_sb[:, NT // 2 : NT, :],
                in_=mask_ps[:, NT // 2 : NT, :],
                func=Act.Copy, bias=0.0, scale=1.0,
            ).then_inc(act_sem, 1)
            # act=3: full bf16 mask in SBUF (DVE E multiply)
            scalar.wait_ge(pe_sem, 3)
            nc.scalar.activation(
                out=presT_t, in_=presT_ps, func=Act.Copy, bias=0.0, scale=1.0,
            ).then_inc(act_sem, 1)
            # act=4: presT in SBUF (PE gathers)

        @block.vector
        def _(vector):
            nc.vector.memset(ones_t, 1)
            vector.drain()
            vector.wait_ge(hr_sem, 16)
            nc.vector.tensor_scalar(
                out=amt_t[:, 0, :], in0=hr_ap, scalar1=15, scalar2=0,
                op0=Alu.subtract, op1=Alu.max,
            )
            nc.vector.tensor_scalar(
                out=amt_t[:, 1, :], in0=hr_ap, scalar1=30, scalar2=0,
                op0=Alu.subtract, op1=Alu.max,
            )
            vector.drain()
            nc.vector.tensor_tensor(
                out=w_t[:, 0, :], in0=ones_t, in1=hr_ap,
                op=Alu.logical_shift_left,
            )
            nc.vector.tensor_tensor(
                out=w_t[:, 1:3, :],
                in0=ones_t[:, None, :].broadcast_to((128, 2, 512)),
                in1=amt_t, op=Alu.logical_shift_left,
            )
            vector.drain()
            nc.vector.tensor_tensor(
                out=w_t[:, :, 0:256], in0=w_t[:, :, 0:256],
                in1=w_t[:, :, 256:512], op=Alu.bitwise_or,
            )
            vector.wait_ge(big_sem, 16)
            # A'[v,j] = [max(r_low[j],1) == v] (row 0 dead: vv row0 = -1);
            # Bm[v,k] = [r_high[k] >= v].  Slotted mid-pack so the PE mask
            # matmuls and ACT's f32->bf16 psum copy hide under the pack tail
            nc.vector.tensor_scalar(
                out=Ap_t, in0=vv_ap[:, 0:IN], scalar1=1.0, scalar2=vom_ap,
                op0=Alu.max, op1=Alu.is_equal,
            )
            nc.vector.tensor_scalar(
                out=Bm_t, in0=vv_ap[:, IN : IN + KSH], scalar1=vom_ap,
                scalar2=None, op0=Alu.is_ge,
            ).then_inc(dve_sem, 1)
            # dve=1: A'/Bm ready -> PE mask matmuls
            nc.vector.tensor_scalar(
                out=oh_t, in0=vv_ap, scalar1=vom_ap, scalar2=pow_ap,
                op0=Alu.is_equal, op1=Alu.mult,
            ).then_inc(dve_sem, 1)
            # dve=2: oh ready
            vector.drain()
            nc.vector.tensor_tensor(
                out=w_t[:, :, 0:128], in0=w_t[:, :, 0:128],
                in1=w_t[:, :, 128:256], op=Alu.bitwise_or,
            )
            vector.drain()
            nc.vector.tensor_reduce(
                out=packed_t, in_=w_t[:, :, 0:128], axis=mybir.AxisListType.X,
                op=Alu.bitwise_or,
            )
            vector.drain()
            nc.vector.tensor_copy(out=packhi_t, in_=packed_t[64:128, :])
            vector.drain()
            nc.vector.tensor_tensor(
                out=packed_t[0:64, :], in0=packed_t[0:64, :], in1=packhi_t,
                op=Alu.bitwise_or,
            )
            vector.drain()
            # extract the 33 presence columns; bitwise ops stay integer-typed
            nc.vector.tensor_tensor(
                out=p33u_t[:, 0:16],
                in0=packed_t[0:64, 0:1].broadcast_to((64, 16)),
                in1=msk_ap[:, 0:16], op=Alu.bitwise_and,
            )
            nc.vector.tensor_tensor(
                out=p33u_t[:, 16:31],
                in0=packed_t[0:64, 1:2].broadcast_to((64, 15)),
                in1=msk_ap[:, 16:31], op=Alu.bitwise_and,
            )
            nc.vector.tensor_tensor(
                out=p33u_t[:, 31:33],
                in0=packed_t[0:64, 2:3].broadcast_to((64, 2)),
                in1=msk_ap[:, 31:33], op=Alu.bitwise_and,
            )
            vector.drain()
            nc.vector.tensor_copy(out=p33_t, in_=p33u_t).then_inc(dve_sem, 1)
            # dve=3: p33 ready -> PE transpose
            vector.wait_ge(act_sem, 3)
            vector.wait_ge(dir_sem, 16)
            nc.vector.tensor_mul(out=E_t, in0=mask_sb, in1=dirT_ap).then_inc(
                dve_sem, 1
            )
            # dve=4: E ready
            vector.wait_ge(pe_sem, 4)
            nc.vector.tensor_mul(
                out=xlT_t[:, 0 : NT // 2, :], in0=xT_ap[:, 0 : NT // 2, :],
                in1=plT_ps[:, 0 : NT // 2, :],
            ).then_inc(dve_sem, 1)
            # dve=5: xlT half 1 -> PE main matmul h1
            vector.wait_ge(pe_sem, 5)
            nc.vector.tensor_mul(
                out=xlT_t[:, NT // 2 : NT, :], in0=xT_ap[:, NT // 2 : NT, :],
                in1=plT_ps[:, NT // 2 : NT, :],
            ).then_inc(dve_sem, 1)
            # dve=6: xlT half 2 -> PE main matmul h2
            vector.wait_ge(pe_sem, 6)
            nc.vector.scalar_tensor_tensor(
                out=z_t, in0=Y_ps, scalar=cs_ap,
                in1=cb_ap.broadcast_to((KSH, B)),
                op0=Alu.mult, op1=Alu.add,
            )
            vector.drain()
            nc.vector.tensor_mul(out=outT_t, in0=omT_ps, in1=z_t).then_inc(
                dve_sem, 1
            )
            # dve=7: outT ready (sync issues the store)

        @block.tensor
        def _(tensor):
            tensor.wait_ge(dve_sem, 1)
            for t in range(NT):
                ins = nc.tensor.matmul(
                    mask_ps[:, t, :], Ap_t[:, t * 128 : (t + 1) * 128], Bm_t,
                )
                if t == NT // 2 - 1:
                    ins.then_inc(pe_sem, 1)  # pe=1: mask half 1 -> ACT
            ins.then_inc(pe_sem, 1)          # pe=2: mask half 2 -> ACT
            tensor.wait_ge(dve_sem, 3)
            tensor.wait_ge(act_sem, 1)
            nc.tensor.transpose(presT_ps, p33_t, ident_t).then_inc(pe_sem, 1)
            # pe=3: presT_ps ready (ACT copies it to SBUF)
            tensor.wait_ge(act_sem, 4)
            for t in range(NT):
                ins = nc.tensor.matmul(
                    plT_ps[:, t, :],
                    oh_t[:, t * 128 : (t + 1) * 128],
                    presT_t,
                )
                if t == NT // 2 - 1:
                    ins.then_inc(pe_sem, 1)  # pe=4: plT half 1
            nc.tensor.matmul(
                omT_ps, oh_t[:, IN : IN + KSH], presT_t,
            ).then_inc(pe_sem, 1)            # pe=5: plT half 2 + om
            tensor.wait_ge(dve_sem, 5)
            for t in range(NT // 2):
                nc.tensor.matmul(
                    Y_ps, E_t[:, t, :], xlT_t[:, t, :],
                    start=(t == 0), stop=False,
                )
            tensor.wait_ge(dve_sem, 6)
            for t in range(NT // 2, NT):
                ins = nc.tensor.matmul(
                    Y_ps, E_t[:, t, :], xlT_t[:, t, :],
                    start=False, stop=(t == NT - 1),
                )
            ins.then_inc(pe_sem, 1)          # pe=6: Y ready

    return nc


def _host_tables():
    import ml_dtypes

    bf = ml_dtypes.bfloat16
    ident = np.eye(64, dtype=bf)
    # column c <-> value c: c 0..15 = w0 bits 0..15, c 16..30 = w1 bits
    # 1..15, c 31..32 = w2 bits 1..2
    bits = np.concatenate(
        [np.arange(0, 16), np.arange(1, 16), np.arange(1, 3)]
    )
    masks = np.zeros((64, 34), np.uint16)
    masks[:, 0:NV] = (np.uint16(1) << bits.astype(np.uint16))[None, :]
    vom = np.arange(NV, dtype=np.float32)[:, None]
    powv = (2.0 ** -bits.astype(np.float32))[:, None]
    return ident, masks, vom, powv


def _prep_in_maps(inputs):
    """Host-side sharding: layout / dtype transforms only, no arithmetic."""
    import ml_dtypes

    bf = ml_dtypes.bfloat16
    x = np.ascontiguousarray(np.asarray(inputs["x"], dtype=np.float32))
    hr = np.ascontiguousarray(np.asarray(inputs["hidden_rank"], dtype=np.int32))
    r_low = np.asarray(inputs["r_low"], dtype=np.int32)
    r_high = np.asarray(inputs["r_high"], dtype=np.int32)
    direction = np.asarray(inputs["direction"], dtype=np.float32)
    cscale_b = np.asarray(inputs["cscale_b"], dtype=np.float32)
    cbias_b = np.asarray(inputs["cbias_b"], dtype=np.float32)

    # partition p = h*64 + b, free = s: hr2[h*64+b, s] = hr[b, h*512+s]
    hr2 = hr.reshape(B, 2, 512).transpose(1, 0, 2).reshape(128, 512)
    hrp = hr2.astype(np.uint16).view(np.float32)  # [128, 256]

    xT3 = x.T.reshape(NT, 128, B).transpose(1, 0, 2)  # [128, NT, B]
    xTp = xT3.reshape(128, -1).astype(bf).view(np.float32)

    rlowf = r_low.astype(np.float32)
    rhighf = r_high.astype(np.float32)
    ident, masks, vom, powv = _host_tables()

    big = np.zeros((128, GW), np.float32)
    big[:, G_X : G_X + 256] = xTp
    big[0:64, G_MSK : G_MSK + 17] = masks.view(np.float32)
    big[0:64, G_IDT : G_IDT + 32] = ident.view(np.float32)
    big[0:NV, G_VOM : G_VOM + 1] = vom
    big[0:NV, G_POW : G_POW + 1] = powv
    vv = np.zeros((NV, IN + KSH), bf)
    vv[0:NV, 0:IN] = rlowf[None, :].astype(bf)
    vv[0, 0:IN] = -1.0  # kills the pl gather's value-0 row (r_low != 0)

    in_maps = []
    for c in range(NCORES):
        sl = slice(c * KSH, (c + 1) * KSH)
        rh = rhighf[sl]
        bigc = big.copy()
        bigc[:, G_CS] = cscale_b[sl]
        bigc[:, G_CB] = cbias_b[sl]
        vvc = vv.copy()
        vvc[:, IN : IN + KSH] = rh[None, :].astype(bf)
        bigc[0:NV, G_VV : G_VV + 576] = vvc.view(np.float32)
        dirc = (
            direction[sl, :].T.reshape(NT, 128, KSH).transpose(1, 0, 2)
            .reshape(128, -1).astype(bf).view(np.float32)
        )
        in_maps.append({"hrp": hrp, "big": bigc, "dir": dirc})
    return in_maps


def _run(inputs, trace=False, **kw):
    from concourse.bass_utils import run_bass_kernel_spmd

    if "nc" not in _cached:
        _cached["nc"] = _build_nc()
    nc = _cached["nc"]
    in_maps = _prep_in_maps(inputs)
    res = run_bass_kernel_spmd(
        nc, in_maps, core_ids=list(range(NCORES)), trace=trace, **kw
    )
    out = np.concatenate([np.asarray(r["out"]).T for r in res.results], axis=1)
    return out.astype(np.float32), res


def kernel(**inputs):
    out, _ = _run(inputs, trace=False)
    return out


# revision 24
# speedup vs baseline: 1.1679x; 1.1679x over previous
"""Trainium2 Bass kernel for nn_AMMaskedLinear (v6).

Math: the reference's per-sample weight mask is separable:
    present[b,v] = any_j(hidden_rank[b,j] == v)            (v in 0..32)
    pl[b,i] = present[b, r_low[i]]  & (r_low[i]  != 0)
    om[b,o] = present[b, r_high[o]]
    E[j,k]  = (r_high[k] >= max(r_low[j], 1)) * direction^T[j,k]
    Y[k,b]  = sum_j E[j,k] * pl[j,b] * x[j,b]
    out[k,b] = om[k,b] * (cscale_b[k] * Y[k,b] + cbias_b[k])

v6 over v5 (23.2us):
  - the E mask moves off DVE onto the otherwise-idle PE as a rank-33
    one-hot matmul into a BF16 psum (v4's f32 psum made the E multiply
    read at half rate): mask[j,k] = sum_v [max(r_low[j],1)==v]*[v<=
    r_high[k]].  The A'/Bm table builds slot into the pack right after
    big lands, so the PE mask matmuls fully overlap the pack tail and
    replace DVE's 1.3us IS_LE with ~0.65us of table ops.
  - w1/w2 shifts fused into one [128,2,512] tensor_tensor.
  - identity copy moved to ACT (was a 160ns DVE op + drain).
  - epilogue back to z = cs*Y + cb (stt) then out = z*om (one TT).

Presence pack (HW-verified shift-overflow-to-zero; probe: u16 << s == 0
for s >= 16): w0 = 1<<hr (values 0..15, no clamp), w1 = 1<<max(hr-15,0)
(16..30), w2 = 1<<max(hr-30,0) (31..32); natural value<->column c == v.
vv row 0 holds -1 on the r_low side so the pl gather's (r_low != 0)
comes from the table contents; both gathers contract rows 0:33.

Timing floors (measured): any 128-partition DMA's semaphore is gated by
SDMA engine 15's wake at ~10-11us from NEFF start; sync's qSP HWDGE ring
is serviced before qAct's, so all loads stay on sync.
"""

import numpy as np

B, IN, OUT, D = 64, 1024, 1024, 32
NCORES = 8
KSH = OUT // NCORES  # 128 outputs per core
NT = IN // 128       # 8 contraction tiles
NV = 33              # rank values 0..32

HRW = 256            # hrp [128, 256] f32 = [128, 512] u16

# big [128, GW] f32 — chain 2 on sync (tables + x; dirT split out so
# this chain lands ~1.3us earlier; dirT is not needed until E)
G_X = 0              # [128, 256]  xT bf16 [128, 8, 64]
G_CS = 256           # [128, 1]    cscale_b shard f32
G_CB = 257           # [128, 1]    cbias_b shard f32
G_MSK = 258          # [64, 17]    u16 extraction masks [64, 34]
G_IDT = 275          # [64, 32]    bf16 identity [64, 64]
G_VOM = 307          # [33, 1]     value-iota f32 (vom[c] = c)
G_POW = 308          # [33, 1]     2^-bit compensation factors f32
G_VV = 309           # [33, 576]   bf16 r_low||r_high bcast [33, 1152]
GW = 885
DW = 512             # dir [128, 512] f32 = dirT bf16 [128, 8, 128], chain 3

_cached = {}


def _build_nc():
    import contextlib

    import concourse.bass as bass
    import concourse.mybir as mybir

    f32 = mybir.dt.float32
    bf16 = mybir.dt.bfloat16
    u16 = mybir.dt.uint16
    Alu = mybir.AluOpType
    Act = mybir.ActivationFunctionType

    nc = bass.Bass()

    hrp_h = nc.declare_dram_parameter("hrp", [128, HRW], f32, isOutput=False)
    big_h = nc.declare_dram_parameter("big", [128, GW], f32, isOutput=False)
    dir_h = nc.declare_dram_parameter("dir", [128, DW], f32, isOutput=False)
    out_h = nc.declare_dram_parameter("out", [KSH, B], f32, isOutput=True)

    ctx = contextlib.ExitStack()

    def sb(name, shape, dt=f32):
        return ctx.enter_context(nc.sbuf_tensor(name, shape, dt))[:]

    def ps(name, shape, dt=f32):
        return ctx.enter_context(nc.psum_tensor(name, shape, dt))[:]

    with ctx:
        hrp_t = sb("hrp_t", [128, HRW])
        big_t = sb("big_t", [128, GW])
        dir_t = sb("dir_t", [128, DW])
        ident_t = sb("ident_t", [64, 64], bf16)

        ones_t = sb("ones_t", [128, 512], u16)
        amt_t = sb("amt_t", [128, 2, 512], u16)
        w_t = sb("w_t", [128, 3, 512], u16)
        packed_t = sb("packed_t", [128, 3], u16)
        packhi_t = sb("packhi_t", [64, 3], u16)
        p33u_t = sb("p33u_t", [64, NV], u16)
        p33_t = sb("p33_t", [64, NV], bf16)
        presT_t = sb("presT_t", [NV, 64], bf16)
        oh_t = sb("oh_t", [NV, IN + KSH], bf16)
        Ap_t = sb("Ap_t", [NV, IN], bf16)
        Bm_t = sb("Bm_t", [NV, KSH], bf16)
        E_t = sb("E_t", [128, NT, KSH], bf16)
        mask_sb = sb("mask_sb", [128, NT, KSH], bf16)
        xlT_t = sb("xlT_t", [128, NT, B], bf16)
        z_t = sb("z_t", [KSH, B])
        outT_t = sb("outT_t", [KSH, B])
        warm_t = sb("warm_t", [128, 1])

        presT_ps = ps("presT_ps", [NV, 64], bf16)
        mask_ps = ps("mask_ps", [128, NT, KSH])
        plT_ps = ps("plT_ps", [128, NT, B])
        omT_ps = ps("omT_ps", [KSH, B])
        Y_ps = ps("Y_ps", [KSH, B])

        hr_ap = hrp_t[:, :].bitcast(u16)                           # [128, 512]
        cs_ap = big_t[:, G_CS : G_CS + 1]
        cb_ap = big_t[:, G_CB : G_CB + 1]
        msk_ap = big_t[0:64, G_MSK : G_MSK + 17].bitcast(u16)      # [64, 34]
        ident_ap = big_t[0:64, G_IDT : G_IDT + 32].bitcast(bf16)
        vom_ap = big_t[0:NV, G_VOM : G_VOM + 1]
        pow_ap = big_t[0:NV, G_POW : G_POW + 1]
        vv_ap = big_t[0:NV, G_VV : G_VV + (IN + KSH) // 2].bitcast(bf16)
        xT_ap = big_t[:, G_X : G_X + 256].bitcast(bf16).rearrange(
            "p (t b) -> p t b", t=NT
        )
        dirT_ap = dir_t[:, :].bitcast(bf16).rearrange(
            "p (t k) -> p t k", t=NT
        )

        hr_sem = ctx.enter_context(nc.semaphore("hr_sem"))
        big_sem = ctx.enter_context(nc.semaphore("big_sem"))
        dir_sem = ctx.enter_context(nc.semaphore("dir_sem"))
        out_sem = ctx.enter_context(nc.semaphore("out_sem"))
        dve_sem = ctx.enter_context(nc.semaphore("dve_sem"))
        pe_sem = ctx.enter_context(nc.semaphore("pe_sem"))
        act_sem = ctx.enter_context(nc.semaphore("act_sem"))
        block = ctx.enter_context(nc.Block())

        @block.sync
        def _(sync):
            # ALL dmas on sync: the qAct ring is serviced after qSP's
            sync.dma_start(out=hrp_t, in_=hrp_h[:, :]).then_inc(hr_sem, 16)
            sync.dma_start(out=big_t, in_=big_h[:, :]).then_inc(big_sem, 16)
            sync.dma_start(out=dir_t, in_=dir_h[:, :]).then_inc(dir_sem, 16)
            sync.wait_ge(dve_sem, 7)
            sync.dma_start(out=out_h[:, :], in_=outT_t).then_inc(out_sem, 16)
            sync.wait_ge(out_sem, 16)  # required: NEFF exit must not race the store


        @block.scalar
        def _(scalar):
            # warm the activation table in the DMA shadow (content irrelevant)
            nc.scalar.activation(
                out=warm_t, in_=big_t[:, 0:1], func=Act.Identity,
                bias=0.0, scale=1.0,
            )
            scalar.wait_ge(big_sem, 16)
            nc.scalar.activation(
                out=ident_t, in_=ident_ap, func=Act.Copy, bias=0.0, scale=1.0,
            ).then_inc(act_sem, 1)
            # act=1: ident in SBUF (PE transpose)
            scalar.wait_ge(pe_sem, 1)
            nc.scalar.activation(
                out=mask_sb, in_=mask_ps, func=Act.Copy, bias=0.0, scale=1.0,
            ).then_inc(act_sem, 1)
            # act=2: bf16 mask in SBUF (DVE E multiply)
            scalar.wait_ge(pe_sem, 2)
            nc.scalar.activation(
                out=presT_t, in_=presT_ps, func=Act.Copy, bias=0.0, scale=1.0,
            ).then_inc(act_sem, 1)
            # act=3: presT in SBUF (PE gathers)

        @block.vector
        def _(vector):
            nc.vector.memset(ones_t, 1)
            vector.drain()
            vector.wait_ge(hr_sem, 16)
            nc.vector.tensor_scalar(
                out=amt_t[:, 0, :], in0=hr_ap, scalar1=15, scalar2=0,
                op0=Alu.subtract, op1=Alu.max,
            )
            nc.vector.tensor_scalar(
                out=amt_t[:, 1, :], in0=hr_ap, scalar1=30, scalar2=0,
                op0=Alu.subtract, op1=Alu.max,
            )
            vector.drain()
            nc.vector.tensor_tensor(
                out=w_t[:, 0, :], in0=ones_t, in1=hr_ap,
                op=Alu.logical_shift_left,
            )
            nc.vector.tensor_tensor(
                out=w_t[:, 1:3, :],
                in0=ones_t[:, None, :].broadcast_to((128, 2, 512)),
                in1=amt_t, op=Alu.logical_shift_left,
            )
            vector.drain()
            nc.vector.tensor_tensor(
                out=w_t[:, :, 0:256], in0=w_t[:, :, 0:256],
                in1=w_t[:, :, 256:512], op=Alu.bitwise_or,
            )
            vector.wait_ge(big_sem, 16)
            # A'[v,j] = [max(r_low[j],1) == v] (row 0 dead: vv row0 = -1);
            # Bm[v,k] = [r_high[k] >= v].  Slotted mid-pack so the PE mask
            # matmuls and ACT's f32->bf16 psum copy hide under the pack tail
            nc.vector.tensor_scalar(
                out=Ap_t, in0=vv_ap[:, 0:IN], scalar1=1.0, scalar2=vom_ap,
                op0=Alu.max, op1=Alu.is_equal,
            )
            nc.vector.tensor_scalar(
                out=Bm_t, in0=vv_ap[:, IN : IN + KSH], scalar1=vom_ap,
                scalar2=None, op0=Alu.is_ge,
            ).then_inc(dve_sem, 1)
            # dve=1: A'/Bm ready -> PE mask matmuls
            vector.drain()
            nc.vector.tensor_tensor(
                out=w_t[:, :, 0:128], in0=w_t[:, :, 0:128],
                in1=w_t[:, :, 128:256], op=Alu.bitwise_or,
            )
            vector.drain()
            nc.vector.tensor_reduce(
                out=packed_t, in_=w_t[:, :, 0:128], axis=mybir.AxisListType.X,
                op=Alu.bitwise_or,
            )
            vector.drain()
            nc.vector.tensor_copy(out=packhi_t, in_=packed_t[64:128, :])
            vector.drain()
            nc.vector.tensor_tensor(
                out=packed_t[0:64, :], in0=packed_t[0:64, :], in1=packhi_t,
                op=Alu.bitwise_or,
            )
            vector.drain()
            # extract the 33 presence columns; bitwise ops stay integer-typed
            nc.vector.tensor_tensor(
                out=p33u_t[:, 0:16],
                in0=packed_t[0:64, 0:1].broadcast_to((64, 16)),
                in1=msk_ap[:, 0:16], op=Alu.bitwise_and,
            )
            nc.vector.tensor_tensor(
                out=p33u_t[:, 16:31],
                in0=packed_t[0:64, 1:2].broadcast_to((64, 15)),
                in1=msk_ap[:, 16:31], op=Alu.bitwise_and,
            )
            nc.vector.tensor_tensor(
                out=p33u_t[:, 31:33],
                in0=packed_t[0:64, 2:3].broadcast_to((64, 2)),
                in1=msk_ap[:, 31:33], op=Alu.bitwise_and,
            )
            vector.drain()
            nc.vector.tensor_copy(out=p33_t, in_=p33u_t).then_inc(dve_sem, 1)
            # dve=2: p33 ready -> PE transpose
            nc.vector.tensor_scalar(
                out=oh_t, in0=vv_ap, scalar1=vom_ap, scalar2=pow_ap,
                op0=Alu.is_equal, op1=Alu.mult,
            ).then_inc(dve_sem, 1)
            # dve=3: oh ready -> PE gathers (with act=3)
            vector.wait_ge(act_sem, 2)
            vector.wait_ge(dir_sem, 16)
            nc.vector.tensor_mul(out=E_t, in0=mask_sb, in1=dirT_ap).then_inc(
                dve_sem, 1
            )
            # dve=4: E ready
            vector.wait_ge(pe_sem, 3)
            nc.vector.tensor_mul(
                out=xlT_t[:, 0 : NT // 2, :], in0=xT_ap[:, 0 : NT // 2, :],
                in1=plT_ps[:, 0 : NT // 2, :],
            ).then_inc(dve_sem, 1)
            # dve=5: xlT half 1 -> PE main matmul h1
            vector.wait_ge(pe_sem, 4)
            nc.vector.tensor_mul(
                out=xlT_t[:, NT // 2 : NT, :], in0=xT_ap[:, NT // 2 : NT, :],
                in1=plT_ps[:, NT // 2 : NT, :],
            ).then_inc(dve_sem, 1)
            # dve=6: xlT half 2 -> PE main matmul h2
            vector.wait_ge(pe_sem, 5)
            nc.vector.scalar_tensor_tensor(
                out=z_t, in0=Y_ps, scalar=cs_ap,
                in1=cb_ap.broadcast_to((KSH, B)),
                op0=Alu.mult, op1=Alu.add,
            )
            vector.drain()
            nc.vector.tensor_mul(out=outT_t, in0=omT_ps, in1=z_t).then_inc(
                dve_sem, 1
            )
            # dve=7: outT ready (sync issues the store)

        @block.tensor
        def _(tensor):
            tensor.wait_ge(dve_sem, 1)
            for t in range(NT):
                ins = nc.tensor.matmul(
                    mask_ps[:, t, :], Ap_t[:, t * 128 : (t + 1) * 128], Bm_t,
                )
            ins.then_inc(pe_sem, 1)          # pe=1: mask ready -> DVE E
            tensor.wait_ge(dve_sem, 2)
            tensor.wait_ge(act_sem, 1)
            nc.tensor.transpose(presT_ps, p33_t, ident_t).then_inc(pe_sem, 1)
            # pe=2: presT_ps ready (ACT copies it to SBUF)
            tensor.wait_ge(dve_sem, 3)
            tensor.wait_ge(act_sem, 3)
            for t in range(NT):
                ins = nc.tensor.matmul(
                    plT_ps[:, t, :],
                    oh_t[:, t * 128 : (t + 1) * 128],
                    presT_t,
                )
                if t == NT // 2 - 1:
                    ins.then_inc(pe_sem, 1)  # pe=3: plT half 1
            nc.tensor.matmul(
                omT_ps, oh_t[:, IN : IN + KSH], presT_t,
            ).then_inc(pe_sem, 1)            # pe=4: plT half 2 + om
            tensor.wait_ge(dve_sem, 5)
            for t in range(NT // 2):
                nc.tensor.matmul(
                    Y_ps, E_t[:, t, :], xlT_t[:, t, :],
                    start=(t == 0), stop=False,
                )
            tensor.wait_ge(dve_sem, 6)
            for t in range(NT // 2, NT):
                ins = nc.tensor.matmul(
                    Y_ps, E_t[:, t, :], xlT_t[:, t, :],
                    start=False, stop=(t == NT - 1),
                )
            ins.then_inc(pe_sem, 1)          # pe=5: Y ready

    return nc


def _host_tables():
    import ml_dtypes

    bf = ml_dtypes.bfloat16
    ident = np.eye(64, dtype=bf)
    # column c <-> value c: c 0..15 = w0 bits 0..15, c 16..30 = w1 bits
    # 1..15, c 31..32 = w2 bits 1..2
    bits = np.concatenate(
        [np.arange(0, 16), np.arange(1, 16), np.arange(1, 3)]
    )
    masks = np.zeros((64, 34), np.uint16)
    masks[:, 0:NV] = (np.uint16(1) << bits.astype(np.uint16))[None, :]
    vom = np.arange(NV, dtype=np.float32)[:, None]
    powv = (2.0 ** -bits.astype(np.float32))[:, None]
    return ident, masks, vom, powv


def _prep_in_maps(inputs):
    """Host-side sharding: layout / dtype transforms only, no arithmetic."""
    import ml_dtypes

    bf = ml_dtypes.bfloat16
    x = np.ascontiguousarray(np.asarray(inputs["x"], dtype=np.float32))
    hr = np.ascontiguousarray(np.asarray(inputs["hidden_rank"], dtype=np.int32))
    r_low = np.asarray(inputs["r_low"], dtype=np.int32)
    r_high = np.asarray(inputs["r_high"], dtype=np.int32)
    direction = np.asarray(inputs["direction"], dtype=np.float32)
    cscale_b = np.asarray(inputs["cscale_b"], dtype=np.float32)
    cbias_b = np.asarray(inputs["cbias_b"], dtype=np.float32)

    # partition p = h*64 + b, free = s: hr2[h*64+b, s] = hr[b, h*512+s]
    hr2 = hr.reshape(B, 2, 512).transpose(1, 0, 2).reshape(128, 512)
    hrp = hr2.astype(np.uint16).view(np.float32)  # [128, 256]

    xT3 = x.T.reshape(NT, 128, B).transpose(1, 0, 2)  # [128, NT, B]
    xTp = xT3.reshape(128, -1).astype(bf).view(np.float32)

    rlowf = r_low.astype(np.float32)
    rhighf = r_high.astype(np.float32)
    ident, masks, vom, powv = _host_tables()

    big = np.zeros((128, GW), np.float32)
    big[:, G_X : G_X + 256] = xTp
    big[0:64, G_MSK : G_MSK + 17] = masks.view(np.float32)
    big[0:64, G_IDT : G_IDT + 32] = ident.view(np.float32)
    big[0:NV, G_VOM : G_VOM + 1] = vom
    big[0:NV, G_POW : G_POW + 1] = powv
    vv = np.zeros((NV, IN + KSH), bf)
    vv[0:NV, 0:IN] = rlowf[None, :].astype(bf)
    vv[0, 0:IN] = -1.0  # kills the pl gather's value-0 row (r_low != 0)

    in_maps = []
    for c in range(NCORES):
        sl = slice(c * KSH, (c + 1) * KSH)
        rh = rhighf[sl]
        bigc = big.copy()
        bigc[:, G_CS] = cscale_b[sl]
        bigc[:, G_CB] = cbias_b[sl]
        vvc = vv.copy()
        vvc[:, IN : IN + KSH] = rh[None, :].astype(bf)
        bigc[0:NV, G_VV : G_VV + 576] = vvc.view(np.float32)
        dirc = (
            direction[sl, :].T.reshape(NT, 128, KSH).transpose(1, 0, 2)
            .reshape(128, -1).astype(bf).view(np.float32)
        )
        in_maps.append({"hrp": hrp, "big": bigc, "dir": dirc})
    return in_maps


def _run(inputs, trace=False, **kw):
    from concourse.bass_utils import run_bass_kernel_spmd

    if "nc" not in _cached:
        _cached["nc"] = _build_nc()
    nc = _cached["nc"]
    in_maps = _prep_in_maps(inputs)
    res = run_bass_kernel_spmd(
        nc, in_maps, core_ids=list(range(NCORES)), trace=trace, **kw
    )
    out = np.concatenate([np.asarray(r["out"]).T for r in res.results], axis=1)
    return out.astype(np.float32), res


def kernel(**inputs):
    out, _ = _run(inputs, trace=False)
    return out
